# revision 3
# baseline (speedup 1.0000x reference)
"""Trainium2 Bass kernel for the EdgeAttrs GNN message-passing problem, v3.

Per edge e with src s=edge_index[0,e], dst d=edge_index[1,e]:
    y = [mlp1(x_s) | mlp2(x_d) | mlp3(x_s-x_d) | mlp4(x_s*x_d)]        # 4 x [E,128]
    s = cos_sim(x_s, x_d)                                              # [E,1]
    out = tanh([y | s | edge_attr] @ Wf)                               # [E,128]

The run is wall-clock dominated by host->device transfer over the axon
tunnel, so v4 minimizes wire bytes relative to the baseline (~282MB ->
~171MB) while avoiding collectives (a NEFF containing a collective pays a
nondeterministic 6s-220s load/setup penalty on its first execution):
  * Edges are assigned to cores by src-node range (core = src >> 13), so a
    core's compacted node table (np.unique of its edge endpoints, local
    indices fit int16 for gpsimd.dma_gather) holds at most its own 8192
    src rows plus ~12.7k external dst rows: ~20.1k rows vs ~25.9k for the
    baseline's contiguous edge sharding. Host permutes edge_attr on the way
    in and inverse-permutes the output rows on the way out. Per-core edge
    counts vary (~16.4k +- 130), so each core is padded to EPC_P = 16896
    edges with dummy edges referencing local row 0 (outputs discarded).
  * Output in f16 (tanh output, so |out|<=1 and f16 adds ~5e-4 abs error):
    halves both the result download AND the zero donation buffers PJRT
    uploads for custom-call outputs.
  * Index tensors are sent 16 partitions wide and replicated to the
    128-partition wrapped layout on device (the DMA-gather layout needs the
    [16, n] block tiled 8x down the partition dim).
"""

import numpy as np

N_NODES = 65536
E_TOTAL = 131072
D = 256          # node feature dim
O = 128          # mlp output dim
PEA = 32         # edge_attr dim
NCORES = 8
EPC_P = 16896               # padded edges per core (max observed ~16512)
TBL = 20992                 # padded per-core node-table rows (max obs ~20128)
GG = 512                    # edges per dma_gather / compute tile

_CACHE = {}


def _build_program(epc, tbl):
    import concourse.tile as tile
    from concourse import bacc, mybir

    f16 = mybir.dt.float16
    f32 = mybir.dt.float32
    i16 = mybir.dt.int16
    Relu = mybir.ActivationFunctionType.Relu
    Tanh = mybir.ActivationFunctionType.Tanh

    n_g = epc // GG
    nix = epc // 16

    # dma_gather emits one descriptor per gathered row; the SWDGE ring
    # carveout defaults to 1024 descriptor slots, too small for 512-row
    # gathers (several in flight). 65536 B/partition = 4096 slots.
    nc = bacc.Bacc(
        "TRN2",
        target_bir_lowering=False,
        debug=False,
        dynamic_dma_scratch_size=65536,
    )

    xt = nc.dram_tensor("xt", [tbl, D], f16, kind="ExternalInput")
    idxp = nc.dram_tensor("idxp", [16, 2 * nix], i16, kind="ExternalInput")
    eat = nc.dram_tensor("eat", [PEA, epc], f16, kind="ExternalInput")
    wpk = nc.dram_tensor("wpk", [28, 128, 128], f16, kind="ExternalInput")
    wfe = nc.dram_tensor("wfe", [PEA, O], f16, kind="ExternalInput")
    wfs = nc.dram_tensor("wfs", [1, O], f16, kind="ExternalInput")
    out = nc.dram_tensor("out", [O, epc], f16, kind="ExternalOutput")

    with tile.TileContext(nc) as tc:
        with (
            tc.tile_pool(name="const", bufs=1) as cpool,
            tc.tile_pool(name="gath", bufs=2) as gpool,
            tc.tile_pool(name="work", bufs=3) as wpool,
            tc.tile_pool(name="yout", bufs=2) as ypool,
            tc.tile_pool(name="small", bufs=2) as spool,
            tc.tile_pool(name="obuf", bufs=3) as opool,
            tc.tile_pool(name="psA", bufs=2, space="PSUM") as pA,
            tc.tile_pool(name="psB", bufs=2, space="PSUM") as pB,
            tc.tile_pool(name="psO", bufs=2, space="PSUM") as pO,
            tc.tile_pool(name="psC", bufs=2, space="PSUM") as pC,
        ):
            # ---- constants, loaded once ----
            w_sb = cpool.tile([128, 28, 128], f16)
            for i in range(28):
                nc.sync.dma_start(out=w_sb[:, i, :], in_=wpk[i])
            wfe_sb = cpool.tile([PEA, O], f16)
            nc.sync.dma_start(out=wfe_sb[:], in_=wfe[:])
            wfs_sb = cpool.tile([1, O], f16)
            nc.sync.dma_start(out=wfs_sb[:], in_=wfs[:])
            ones_sb = cpool.tile([128, 1], f16)
            nc.vector.memset(ones_sb[:], 1.0)
            # indices: [16, 2*nix] input replicated into the 128-partition
            # wrapped layout (edge i at partition i%16, column i//16).
            idxs = cpool.tile([128, 2 * nix], i16)
            for k in range(8):
                nc.sync.dma_start(out=idxs[16 * k:16 * (k + 1), :], in_=idxp[:])

            relu_rr = 0  # round-robin relu copies between ACT and DVE

            for g in range(n_g):
                c0 = g * (GG // 16)
                c1 = (g + 1) * (GG // 16)
                sgT = gpool.tile([128, 2, GG], f16, tag="sg")
                dgT = gpool.tile([128, 2, GG], f16, tag="dg")
                nc.gpsimd.dma_gather(
                    sgT[:], xt[:], idxs[:, c0:c1], GG, GG, D, transpose=True
                )
                nc.gpsimd.dma_gather(
                    dgT[:], xt[:], idxs[:, nix + c0:nix + c1], GG, GG, D,
                    transpose=True
                )

                eg = g * GG
                sg3 = sgT[:]
                dg3 = dgT[:]
                dif = wpool.tile([128, 2, GG], f16, tag="dif")
                prd = wpool.tile([128, 2, GG], f16, tag="prd")
                sqs = wpool.tile([128, 2, GG], f16, tag="sqs")
                sqd = wpool.tile([128, 2, GG], f16, tag="sqd")
                nc.vector.tensor_sub(dif[:], sg3, dg3)
                nc.vector.tensor_mul(prd[:], sg3, dg3)
                nc.vector.tensor_mul(sqs[:], sg3, sg3)
                nc.vector.tensor_mul(sqd[:], dg3, dg3)

                # cosine-similarity reductions over the feature dim:
                # psum rows 0/32/64 = [sum(s*d), sum(s^2), sum(d^2)]
                # (matmul outputs must start at partition 0, 32 or 64)
                pc = pC.tile([65, GG], f32, tag="pc")
                for h in range(2):
                    st, sp = (h == 0), (h == 1)
                    nc.tensor.matmul(pc[0:1, :], ones_sb[:], prd[:, h, :], start=st, stop=sp)
                    nc.tensor.matmul(pc[32:33, :], ones_sb[:], sqs[:, h, :], start=st, stop=sp)
                    nc.tensor.matmul(pc[64:65, :], ones_sb[:], sqd[:, h, :], start=st, stop=sp)
                # HW constraint: at most one non-scalar PSUM input per DVE op
                ssb = spool.tile([1, GG], f32, tag="ssb")
                nc.vector.tensor_copy(ssb[:], pc[64:65, :])
                nsq = spool.tile([1, GG], f32, tag="nsq")
                nc.vector.tensor_mul(nsq[:], pc[32:33, :], ssb[:])
                nrm = spool.tile([1, GG], f32, tag="nrm")
                nc.scalar.sqrt(nrm[:], nsq[:])
                inv = spool.tile([1, GG], f32, tag="inv")
                nc.vector.reciprocal(inv[:], nrm[:])
                s16 = spool.tile([1, GG], f16, tag="s16")
                nc.vector.tensor_mul(s16[:], pc[0:1, :], inv[:])

                # ---- the 4 two-layer MLPs, all feature-major ----
                ins3 = [sg3, dg3, dif[:], prd[:]]
                ys = []
                for m in range(4):
                    inm = ins3[m]
                    aT = wpool.tile([128, 2, GG], f16, tag="aT")
                    for mo in range(2):
                        pa = pA.tile([128, GG], f32, tag="pa")
                        for h in range(2):
                            nc.tensor.matmul(
                                pa[:],
                                w_sb[:, m * 4 + h * 2 + mo, :],
                                inm[:, h, :],
                                start=(h == 0),
                                stop=(h == 1),
                            )
                        if relu_rr % 2 == 0:
                            nc.scalar.activation(aT[:, mo, :], pa[:], Relu)
                        else:
                            nc.vector.tensor_relu(aT[:, mo, :], pa[:])
                        relu_rr += 1
                    pb = pB.tile([128, GG], f32, tag="pb")
                    for h in range(2):
                        nc.tensor.matmul(
                            pb[:],
                            w_sb[:, 16 + m * 2 + h, :],
                            aT[:, h, :],
                            start=(h == 0),
                            stop=(h == 1),
                        )
                    ym = ypool.tile([128, GG], f16, tag=f"y{m}")
                    if relu_rr % 2 == 0:
                        nc.scalar.activation(ym[:], pb[:], Relu)
                    else:
                        nc.vector.tensor_relu(ym[:], pb[:])
                    relu_rr += 1
                    ys.append(ym)

                # ---- final linear over z = [y1|y2|y3|y4|s|ea] + tanh ----
                ea_sb = spool.tile([PEA, GG], f16, tag="ea")
                nc.sync.dma_start(out=ea_sb[:], in_=eat[:, eg:eg + GG])
                po = pO.tile([128, GG], f32, tag="po")
                for k in range(4):
                    nc.tensor.matmul(po[:], w_sb[:, 24 + k, :], ys[k][:], start=(k == 0), stop=False)
                nc.tensor.matmul(po[:], wfe_sb[:], ea_sb[:], start=False, stop=False)
                nc.tensor.matmul(po[:], wfs_sb[:], s16[:], start=False, stop=True)
                ot = opool.tile([128, GG], f16, tag="ot")
                nc.scalar.activation(ot[:], po[:], Tanh)
                nc.sync.dma_start(out=out[:, eg:eg + GG], in_=ot[:])

    nc.compile()
    return nc


def get_program(epc=EPC_P, tbl=TBL):
    key = (epc, tbl)
    if key not in _CACHE:
        _CACHE[key] = _build_program(epc, tbl)
    return _CACHE[key]


def _pack_weights(inputs):
    f16 = np.float16
    wpk = np.zeros((28, 128, 128), f16)
    for m, name in enumerate(["1", "2", "3", "4"]):
        Wa = np.asarray(inputs[f"W{name}a"], np.float32)
        Wb = np.asarray(inputs[f"W{name}b"], np.float32)
        for h in range(2):
            for mo in range(2):
                wpk[m * 4 + h * 2 + mo] = Wa[h * 128:(h + 1) * 128, mo * 128:(mo + 1) * 128]
            wpk[16 + m * 2 + h] = Wb[h * 128:(h + 1) * 128, :]
    Wf = np.asarray(inputs["Wf"], np.float32)
    for k in range(4):
        wpk[24 + k] = Wf[k * 128:(k + 1) * 128, :]
    wfe = np.ascontiguousarray(Wf[513:545]).astype(f16)
    wfs = np.ascontiguousarray(Wf[512:513]).astype(f16)
    return wpk, wfe, wfs


def _wrap_idx16(local_idx):
    """[n] int -> [16, n/16] int16 (edge i at partition i%16, column i//16)."""
    n = local_idx.shape[0]
    assert n % 16 == 0
    return np.ascontiguousarray(local_idx.reshape(n // 16, 16).T.astype(np.int16))


def _prep_core_inputs(x, src, dst, ea_shard, wpk, wfe, wfs, tbl, epc_p):
    """Build one core's input map from its (real, unpadded) edge shard."""
    n_real = src.shape[0]
    assert n_real <= epc_p, (n_real, epc_p)
    uniq, inv = np.unique(np.concatenate([src, dst]), return_inverse=True)
    assert uniq.size <= tbl, (uniq.size, tbl)
    # rows >= uniq.size are never indexed by the gather, so no need to zero
    xt = np.empty((tbl, x.shape[1]), np.float16)
    xt[:uniq.size] = x[uniq]
    # pad to epc_p with dummy edges on local row 0 (outputs discarded)
    isrc = np.zeros(epc_p, np.int64)
    idst = np.zeros(epc_p, np.int64)
    isrc[:n_real] = inv[:n_real]
    idst[:n_real] = inv[n_real:]
    eat = np.zeros((PEA, epc_p), np.float16)
    eat[:, :n_real] = ea_shard.astype(np.float16).T
    return {
        "xt": xt,
        "idxp": np.concatenate([_wrap_idx16(isrc), _wrap_idx16(idst)], axis=1),
        "eat": eat,
        "wpk": wpk,
        "wfe": wfe,
        "wfs": wfs,
    }


def kernel(**inputs):
    from concourse.bass_utils import run_bass_kernel_spmd

    x = np.asarray(inputs["x"], np.float32)
    ei = np.asarray(inputs["edge_index"])
    ea = np.asarray(inputs["edge_attr"], np.float32)
    E = ei.shape[1]

    nc = get_program()
    wpk, wfe, wfs = _pack_weights(inputs)
    xf16 = x.astype(np.float16)

    src = np.asarray(ei[0], np.int64)
    dst = np.asarray(ei[1], np.int64)
    # edge -> core by src-node range; stable order keeps cores contiguous
    assign = src >> 13
    order = np.argsort(assign, kind="stable")
    counts = np.bincount(assign, minlength=NCORES)
    assert counts.max() <= EPC_P, counts
    starts = np.concatenate([[0], np.cumsum(counts)])

    in_maps = []
    for c in range(NCORES):
        eidx = order[starts[c]:starts[c + 1]]
        in_maps.append(
            _prep_core_inputs(
                xf16, src[eidx], dst[eidx], ea[eidx],
                wpk, wfe, wfs, TBL, EPC_P,
            )
        )

    res = run_bass_kernel_spmd(nc, in_maps, list(range(NCORES)))
    big = np.concatenate(
        [res.results[c]["out"][:, :counts[c]] for c in range(NCORES)], axis=1)
    out = np.empty((O, E), np.float32)
    out[:, order] = big
    return np.ascontiguousarray(out.T)  # [E, O]
